# revision 56
# baseline (speedup 1.0000x reference)
"""Trainium2 Bass kernel for nn_BlockDirectTwice (dual-branch transformer block).

Sharding: data-parallel over batch. 8 batch elements -> 8 NeuronCores; every
core runs the full block (two LN+MHA branches, blend, LN, MLP, residuals) on
its own [S, D] slab. No collectives.

Numerics: matmuls in bf16 (fp32 PSUM accumulation); LayerNorm statistics,
softmax denominators and residual stream kept in fp32.
"""

import numpy as np
import ml_dtypes

B, S, D, H, DFF = 8, 1024, 768, 12, 3072
HD = D // H          # 64
P = 128
KD = D // P          # 6  K-subtiles over D
KF = DFF // P        # 24 K-subtiles over DFF
NT = S // P          # 8  token tiles
NPAIR = H // 2       # 6  head pairs
UP, MID = 0.6, 0.4
EPS = 1e-6
N_CORES = 8
ATT_SCALE = 1.0 / np.sqrt(HD)  # 0.125

_CACHE = {}


def _split_cols(n):
    """Split n output columns into <=512 chunks."""
    out, c = [], 0
    while c < n:
        w = min(512, n - c)
        out.append((c, w))
        c += w
    return out


def _build_nc(cfg):
    """Build the per-core Bass program. cfg is a frozenset of feature flags."""
    from contextlib import ExitStack

    import concourse.bass as bass
    import concourse.tile as tile
    from concourse import bacc, mybir

    F32 = mybir.dt.float32
    BF16 = mybir.dt.bfloat16
    AF = mybir.ActivationFunctionType
    ALU = mybir.AluOpType

    has = lambda f: f in cfg
    repeat = 1
    for f in cfg:
        if f.startswith("repeat="):
            repeat = int(f.split("=")[1])

    nc = bacc.Bacc("TRN2", target_bir_lowering=False, debug=False)

    # ---------------- DRAM I/O ----------------
    x_dram = [
        nc.dram_tensor("x0", (S, D), F32, kind="ExternalInput"),
        nc.dram_tensor("x1", (S, D), F32, kind="ExternalInput"),
    ]
    w_dram = {}
    for br in (0, 1):
        for nm in ("wq", "wk", "wv", "wo"):
            w_dram[(br, nm)] = nc.dram_tensor(f"a{br}_{nm}", (D, D), BF16,
                                              kind="ExternalInput")
    fc1_dram = nc.dram_tensor("fc1_w", (D, DFF), BF16, kind="ExternalInput")
    fc2_dram = nc.dram_tensor("fc2_w", (DFF, D), BF16, kind="ExternalInput")

    # optional non-trivial params (most are zeros/ones in this problem)
    opt_dram = {}
    for name, shape in [
        ("ln0_g", (D,)), ("ln0_b", (D,)), ("ln1_g", (D,)), ("ln1_b", (D,)),
        ("lnf_g", (D,)), ("lnf_b", (D,)),
        ("fc1_b", (DFF,)), ("fc2_b", (D,)),
        ("a0_bq", (D,)), ("a0_bk", (D,)), ("a0_bv", (D,)), ("a0_bo", (D,)),
        ("a1_bq", (D,)), ("a1_bk", (D,)), ("a1_bv", (D,)), ("a1_bo", (D,)),
    ]:
        if has(name):
            opt_dram[name] = nc.dram_tensor(name, shape, F32, kind="ExternalInput")

    out_dram = nc.dram_tensor("out", (S, D), F32, kind="ExternalOutput")

    def bcast_rows(src_ap, nparts):
        """DRAM row [1, n] (or [n]) -> AP broadcast over nparts partitions."""
        ap = list(src_ap.ap)
        if len(src_ap.shape) == 1:
            ap = [[0, nparts]] + ap
        else:
            ap = [[0, nparts]] + ap[1:]
        return bass.AP(tensor=src_ap.tensor, offset=src_ap.offset, ap=ap)

    with ExitStack() as ctx:
        tc = ctx.enter_context(tile.TileContext(nc))

        sb = ctx.enter_context(tc.tile_pool(name="sb", bufs=1))
        wpool = ctx.enter_context(tc.tile_pool(name="w", bufs=2))
        lnp = ctx.enter_context(tc.tile_pool(name="ln", bufs=2))
        qkp = ctx.enter_context(tc.tile_pool(name="qk", bufs=3))
        xtp = ctx.enter_context(tc.tile_pool(name="xt", bufs=1))
        prp = ctx.enter_context(tc.tile_pool(name="probs", bufs=4))
        outp = ctx.enter_context(tc.tile_pool(name="out", bufs=2))
        psmm = ctx.enter_context(tc.tile_pool(name="psmm", bufs=4, space="PSUM"))
        pssc = ctx.enter_context(tc.tile_pool(name="pssc", bufs=2, space="PSUM"))
        dram = ctx.enter_context(tc.tile_pool(name="dram", bufs=1, space="DRAM"))

        loop_cm = tc.For_i(0, repeat, 1) if repeat > 1 else None
        if loop_cm is not None:
            ctx.enter_context(loop_cm)

        eps_t = sb.tile([P, 1], F32, tag="eps")
        nc.vector.memset(eps_t, EPS)

        # persistent big tensors
        h_tm = sb.tile([P, NT, D], F32, tag="h_tm")
        ctx_all = sb.tile([P, KD, S], BF16, tag="ctx")
        v_aug = sb.tile([P, NT, H * 65], BF16, tag="v")
        fc1_sb = sb.tile([P, KD, DFF], BF16, tag="fc1")
        fc2_sb = sb.tile([P, KF, D], BF16, tag="fc2")
        nc.gpsimd.dma_start(fc1_sb, fc1_dram.ap().rearrange("(ko p) n -> p ko n", p=P))
        nc.gpsimd.dma_start(fc2_sb, fc2_dram.ap().rearrange("(ko p) n -> p ko n", p=P))

        # optional broadcast tiles for per-feature (free-dim) params
        bcast_sb = {}
        for name in ("ln0_g", "ln0_b", "ln1_g", "ln1_b", "lnf_g", "lnf_b",
                     "a0_bv", "a1_bv", "a0_bo", "a1_bo", "fc2_b"):
            if has(name):
                t = sb.tile([P, D], F32, tag=f"bc_{name}")
                nc.gpsimd.dma_start(t, bcast_rows(opt_dram[name].ap(), P))
                bcast_sb[name] = t
        # per-partition bias tiles (feature-major layouts)
        pp_sb = {}
        for name, kk in (("a0_bq", KD), ("a0_bk", KD), ("a1_bq", KD),
                         ("a1_bk", KD), ("fc1_b", KF)):
            if has(name):
                t = sb.tile([P, kk], F32, tag=f"pp_{name}")
                nc.sync.dma_start(t, opt_dram[name].ap().rearrange("(m p) -> p m", p=P))
                pp_sb[name] = t
        for name in ("a0_bq", "a1_bq"):
            if name in pp_sb:  # q is pre-scaled by 1/8; scale its bias too
                nc.vector.tensor_scalar_mul(pp_sb[name], pp_sb[name], float(ATT_SCALE))

        # identity for PE-mode transposes
        from concourse.masks import make_identity
        ident = sb.tile([P, P], BF16, tag="ident")
        make_identity(nc, ident)

        # transposes alternate between the two HWDGE rings
        _ring = [0]

        def dma_T(out_ap, in_ap):
            eng = nc.sync if _ring[0] % 2 == 0 else nc.scalar
            _ring[0] += 1
            if has("notranspose"):  # diagnostic: same bytes, no xbar
                eng.dma_start(out_ap, in_ap)
                return
            eng.dma_start_transpose(out_ap, in_ap)

        def emit_ln(x_f32, xT_dest, t, gname, bname, on_pe=False):
            """LayerNorm x_f32 [P, D] (in-place scratch) -> bf16, transposed into
            xT_dest[:, j, t*128:(t+1)*128]."""
            stats = lnp.tile([P, 3, 6], F32, tag="stats")
            for sg in range(3):
                nc.vector.bn_stats(stats[:, sg, :], x_f32[:, sg * 256:(sg + 1) * 256])
            mv = lnp.tile([P, 2], F32, tag="mv")
            nc.vector.bn_aggr(mv, stats)
            # rstd = 1/sqrt(var+eps), DVE-only (quake init + 2 Newton steps):
            # keeps the ACT table set untouched (exp stays resident).
            rstd = lnp.tile([P, 1], F32, tag="rstd")
            vh = lnp.tile([P, 1], F32, tag="rs_vh")
            nc.vector.tensor_scalar(vh, mv[:, 1:2], EPS, 0.5, ALU.add, ALU.mult)
            yi = lnp.tile([P, 1], mybir.dt.int32, tag="rs_yi")
            # quake seed from the bits of u = var (Newton uses h = u/2)
            nc.vector.tensor_scalar(yi, mv[:, 1:2].bitcast(mybir.dt.int32), 1, None,
                                    ALU.logical_shift_right)
            y0 = lnp.tile([P, 1], F32, tag="rs_y0")
            nc.vector.tensor_scalar(yi, yi, -1, None, ALU.bitwise_xor)
            nc.vector.tensor_scalar(y0.bitcast(mybir.dt.int32), yi, 0x5f3759e0, None,
                                    ALU.add)
            t1 = lnp.tile([P, 1], F32, tag="rs_t1")
            for _ in range(2):
                nc.vector.tensor_tensor(t1, y0, y0, ALU.mult)
                nc.vector.tensor_tensor(t1, t1, vh, ALU.mult)
                nc.vector.tensor_scalar(t1, t1, -1.0, 1.5, ALU.mult, ALU.add)
                nc.vector.tensor_tensor(y0, y0, t1, ALU.mult)
            nc.vector.tensor_copy(rstd, y0)
            nc.vector.tensor_scalar(x_f32, x_f32, mv[:, 0:1], None, ALU.subtract)
            xln = lnp.tile([P, D], BF16, tag="xln")
            if has(gname):
                nc.vector.tensor_scalar_mul(x_f32, x_f32, rstd[:])
                nc.vector.tensor_tensor(x_f32, x_f32, bcast_sb[gname], ALU.mult)
                if has(bname):
                    nc.vector.tensor_tensor(xln, x_f32, bcast_sb[bname], ALU.add)
                else:
                    nc.vector.tensor_copy(xln, x_f32)
            elif has(bname):
                nc.vector.tensor_scalar_mul(x_f32, x_f32, rstd[:])
                nc.vector.tensor_tensor(xln, x_f32, bcast_sb[bname], ALU.add)
            else:
                nc.vector.tensor_scalar_mul(xln, x_f32, rstd[:])
            if on_pe:
                for j in range(KD):
                    pst = psmm.tile([P, 512], F32, tag="mm")
                    nc.tensor.transpose(pst[:, :P].bitcast(BF16)[:, :P], xln[:, j * P:(j + 1) * P], ident)
                    nc.vector.tensor_copy(xT_dest[:, j, t * P:(t + 1) * P],
                                          pst[:, :P].bitcast(BF16)[:, :P])
            else:
                for j in range(KD):
                    dma_T(xT_dest[:, j, t * P:(t + 1) * P], xln[:, j * P:(j + 1) * P])

        def stage_A(br, xT_dest):
            """Load x_br, LN, transpose; accumulate blend into h_tm."""
            g, b = (f"ln{br}_g", f"ln{br}_b")
            for t in range(NT):
                xt = lnp.tile([P, D], F32, tag="x_tm")
                nc.sync.dma_start(xt, x_dram[br].ap()[t * P:(t + 1) * P, :])
                if br == 0:
                    nc.vector.tensor_scalar_mul(h_tm[:, t, :], xt, UP)
                else:
                    nc.vector.scalar_tensor_tensor(h_tm[:, t, :], xt, MID,
                                                   h_tm[:, t, :], ALU.mult, ALU.add)
                emit_ln(xt, xT_dest, t, g, b, on_pe=(br == 0))

        def load_w(br, nm):
            t = wpool.tile([P, KD, D], BF16, tag="w768")
            nc.gpsimd.dma_start(t, w_dram[(br, nm)].ap().rearrange("(ko p) n -> p ko n", p=P))
            return t

        def stage_BC(br, xT, pending_wo=None):
            """V/Q/K projections + attention, with next-pair projection chunks
            interleaved into the attention t-loop so the in-order PE queue
            stays dense while ACT computes exps. Returns a closure that emits
            the wo projection (deferred into the next branch's warmup)."""
            wv = load_w(br, "wv")
            wq = load_w(br, "wq")
            v_view = v_aug[:].rearrange("p t (h c) -> p t h c", c=65)
            nc.vector.memset(v_view[:, :, :, 64:65], 1.0)
            for t in range(NT):
                for c0, cw in _split_cols(D):
                    ps = psmm.tile([P, 512], F32, tag="mm")
                    for k in range(KD):
                        nc.tensor.matmul(
                            ps[:, :cw], lhsT=xT[:, k, t * P:(t + 1) * P],
                            rhs=wv[:, k, c0:c0 + cw],
                            start=(k == 0), stop=(k == KD - 1))
                    nh = cw // HD
                    h0 = c0 // HD
                    src = ps[:, :cw].rearrange("p (h c) -> p h c", c=HD)
                    dst = v_view[:, t, h0:h0 + nh, 0:HD]
                    bias_key = f"a{br}_bv"
                    if bias_key in bcast_sb:
                        bcv = bcast_sb[bias_key][:, c0:c0 + cw].rearrange(
                            "p (h c) -> p h c", c=HD)
                        nc.vector.tensor_tensor(dst, src, bcv, ALU.add)
                    else:
                        nc.vector.tensor_copy(dst, src)
            if pending_wo is not None:
                pending_wo()
            wk = load_w(br, "wk")
            denom_dram = dram.tile([H, S], BF16)
            if has("noattn"):
                nc.vector.memset(ctx_all, 0.25)

            def proj_chunks(pr, qp, kp):
                """12 closures, each one (which, col-chunk) psum of pair pr."""
                chunks = []
                for (which, wt, dest) in (("q", wq, qp), ("k", wk, kp)):
                    for c0, cw in _split_cols(S):
                        def emit(which=which, wt=wt, dest=dest, c0=c0, cw=cw,
                                 last=False, pr=pr):
                            ps = psmm.tile([P, 512], F32, tag="mm")
                            for k in range(KD):
                                nc.tensor.matmul(
                                    ps[:, :cw], lhsT=wt[:, k, pr * P:(pr + 1) * P],
                                    rhs=xT[:, k, c0:c0 + cw],
                                    start=(k == 0), stop=(k == KD - 1))
                            if which == "q":
                                nc.vector.tensor_scalar_mul(
                                    dest[:, c0:c0 + cw], ps[:, :cw], float(ATT_SCALE))
                            else:
                                nc.vector.tensor_copy(dest[:, c0:c0 + cw], ps[:, :cw])
                            bias_key = f"a{br}_b{which}"
                            if bias_key in pp_sb and c0 + cw >= S:
                                nc.vector.tensor_scalar_add(
                                    dest, dest, pp_sb[bias_key][:, pr:pr + 1])
                        chunks.append(emit)
                return chunks

            if not has("noattn"):
                # prime pair 0 (nothing to interleave with yet)
                cur_qp = qkp.tile([P, S], BF16, tag="qpair", bufs=2, name="qp0")
                cur_kp = qkp.tile([P, S], BF16, tag="kpair", bufs=2, name="kp0")
                for ch in proj_chunks(0, cur_qp, cur_kp):
                    ch()
                for pr in range(NPAIR):
                    qp, kp = cur_qp, cur_kp
                    fillers = []
                    if pr + 1 < NPAIR:
                        cur_qp = qkp.tile([P, S], BF16, tag="qpair", bufs=2,
                                          name=f"qp{pr + 1}")
                        cur_kp = qkp.tile([P, S], BF16, tag="kpair", bufs=2,
                                          name=f"kp{pr + 1}")
                        fillers = proj_chunks(pr + 1, cur_qp, cur_kp)
                    nfill = 0
                    for n in range(2):
                        n0 = n * 512
                        ps_c = [psmm.tile([P, 512], F32, tag="mm", name=f"ps_c{hh}")
                                for hh in range(2)]

                        def ctx_step(t, pq):
                            for hh in range(2):
                                h = 2 * pr + hh
                                nc.tensor.matmul(
                                    ps_c[hh][0:65, :],
                                    lhsT=v_aug[:, t, h * 65:(h + 1) * 65],
                                    rhs=pq[:, hh, :],
                                    start=(t == 0), stop=(t == NT - 1))

                        LAG = 2
                        pending = []
                        for t in range(NT):
                            ps_s = pssc.tile([P, 2, 512], F32, tag="sc")
                            for hh in range(2):
                                b0 = hh * HD
                                nc.tensor.matmul(
                                    ps_s[:, hh, :],
                                    lhsT=kp[b0:b0 + HD, t * P:(t + 1) * P],
                                    rhs=qp[b0:b0 + HD, n0:n0 + 512],
                                    start=True, stop=True)
                            pq = prp.tile([P, 2, 512], BF16, tag="probs")
                            nc.scalar.activation(pq, ps_s, AF.Exp)
                            pending.append((t, pq))
                            if len(pending) > LAG:
                                ctx_step(*pending.pop(0))
                            # keep PE dense: one projection chunk of the next
                            # pair after (roughly) every other t-step
                            want = ((n * NT + t + 1) * len(fillers)) // (2 * NT)
                            while nfill < want:
                                fillers[nfill]()
                                nfill += 1
                        for item in pending:
                            ctx_step(*item)
                        for hh in range(2):
                            h = 2 * pr + hh
                            nc.vector.tensor_copy(
                                ctx_all[hh * HD:(hh + 1) * HD, pr, n0:n0 + 512],
                                ps_c[hh][0:HD, :])
                            dstage = lnp.tile([65, 512], BF16, tag="dstage", bufs=1)
                            nc.vector.tensor_copy(dstage[64:65, :], ps_c[hh][64:65, :])
                            nc.gpsimd.dma_start(denom_dram[h:h + 1, n0:n0 + 512],
                                              dstage[64:65, :])
                    while nfill < len(fillers):
                        fillers[nfill]()
                        nfill += 1
            # denominators -> reciprocal -> broadcast
            if not has("noattn"):
                recip_sb = sb.tile([H, S], F32, tag="recip")
                nc.gpsimd.dma_start(recip_sb, denom_dram[:])
                nc.vector.reciprocal_approx_fast(recip_sb, recip_sb)
                recip_dram = dram.tile([H, S], F32)
                nc.sync.dma_start(recip_dram, recip_sb)
                for pr in range(NPAIR):
                    rb = outp.tile([P, S], F32, tag="recipB", bufs=1)
                    for hh in range(2):
                        h = 2 * pr + hh
                        nc.gpsimd.dma_start(rb[hh * HD:(hh + 1) * HD, :],
                                            bcast_rows(recip_dram[h:h + 1, :], HD))
                    nc.vector.tensor_tensor(ctx_all[:, pr, :], ctx_all[:, pr, :],
                                            rb, ALU.mult)

            def emit_wo():
                wo = load_w(br, "wo")
                scale = UP if br == 0 else MID
                _wo_proj(br, wo, scale)

            return emit_wo

        def _wo_proj(br, wo, scale):
            bo_key = f"a{br}_bo"
            for t in range(NT):
                for c0, cw in _split_cols(D):
                    ps = psmm.tile([P, 512], F32, tag="mm")
                    for k in range(KD):
                        nc.tensor.matmul(
                            ps[:, :cw], lhsT=ctx_all[:, k, t * P:(t + 1) * P],
                            rhs=wo[:, k, c0:c0 + cw],
                            start=(k == 0), stop=(k == KD - 1))
                    if bo_key in bcast_sb:
                        tmp = lnp.tile([P, D], F32, tag="wo_tmp")
                        nc.vector.tensor_tensor(tmp[:, :cw], ps[:, :cw],
                                                bcast_sb[bo_key][:, c0:c0 + cw],
                                                ALU.add)
                        nc.vector.scalar_tensor_tensor(
                            h_tm[:, t, c0:c0 + cw], tmp[:, :cw], float(scale),
                            h_tm[:, t, c0:c0 + cw], ALU.mult, ALU.add)
                    else:
                        nc.vector.scalar_tensor_tensor(
                            h_tm[:, t, c0:c0 + cw], ps[:, :cw], float(scale),
                            h_tm[:, t, c0:c0 + cw], ALU.mult, ALU.add)

        # ---------------- emit program ----------------
        xT0 = xtp.tile([P, KD, S], BF16, tag="xT")
        stage_A(0, xT0)
        wo0 = stage_BC(0, xT0)
        xT1 = xtp.tile([P, KD, S], BF16, tag="xT")
        stage_A(1, xT1)
        wo1 = stage_BC(1, xT1, pending_wo=wo0)
        wo1()

        # LNf -> hT
        hT = xtp.tile([P, KD, S], BF16, tag="xT")
        for t in range(NT):
            hc = lnp.tile([P, D], F32, tag="x_tm")
            nc.vector.tensor_copy(hc, h_tm[:, t, :])
            emit_ln(hc, hT, t, "lnf_g", "lnf_b", on_pe=True)

        # MLP: fc1+gelu then fc2+residual, in token chunks of 256
        if has("nomlp"):
            for t in range(NT):
                o_t = outp.tile([P, D], F32, tag="out_t", bufs=1)
                nc.vector.tensor_copy(o_t, h_tm[:, t, :])
                nc.gpsimd.dma_start(out_dram.ap()[t * P:(t + 1) * P, :], o_t)
        for nn in range(4 if not has("nomlp") else 0):
            c0 = nn * 256
            gT = xtp.tile([P, KF, 256], BF16, tag="gT")
            for m in range(KF):
                ps = psmm.tile([P, 512], F32, tag="mm")
                for k in range(KD):
                    nc.tensor.matmul(ps[:, :256], lhsT=fc1_sb[:, k, m * P:(m + 1) * P],
                                     rhs=hT[:, k, c0:c0 + 256],
                                     start=(k == 0), stop=(k == KD - 1))
                bias = pp_sb["fc1_b"][:, m:m + 1] if "fc1_b" in pp_sb else 0.0
                nc.scalar.activation(gT[:, m, :], ps[:, :256], AF.Gelu, bias=bias)
            for tl in range(2):
                t = 2 * nn + tl
                o_t = outp.tile([P, D], F32, tag="out_t", bufs=1)
                for oc0, ocw in _split_cols(D):
                    ps = psmm.tile([P, 512], F32, tag="mm")
                    for k in range(KF):
                        nc.tensor.matmul(
                            ps[:, :ocw], lhsT=gT[:, k, tl * P:(tl + 1) * P],
                            rhs=fc2_sb[:, k, oc0:oc0 + ocw],
                            start=(k == 0), stop=(k == KF - 1))
                    if "fc2_b" in bcast_sb:
                        nc.vector.tensor_tensor(ps[:, :ocw], ps[:, :ocw],
                                                bcast_sb["fc2_b"][:, oc0:oc0 + ocw],
                                                ALU.add)
                    nc.vector.tensor_tensor(o_t[:, oc0:oc0 + ocw], ps[:, :ocw],
                                            h_tm[:, t, oc0:oc0 + ocw], ALU.add)
                nc.gpsimd.dma_start(out_dram.ap()[t * P:(t + 1) * P, :], o_t)

    nc.compile()
    return nc


def _prep_inputs(inputs):
    """Host-side prep: detect trivial params, cast weights to bf16."""
    bf16 = ml_dtypes.bfloat16
    cfg = set()
    arrs = {}
    for name in ("x0", "x1"):
        arrs[name] = np.ascontiguousarray(np.asarray(inputs[name], dtype=np.float32))
    for br in (0, 1):
        for nm in ("wq", "wk", "wv", "wo"):
            key = f"a{br}_{nm}"
            arrs[key] = np.ascontiguousarray(
                np.asarray(inputs[key], dtype=np.float32).astype(bf16))
    arrs["fc1_w"] = np.ascontiguousarray(
        np.asarray(inputs["fc1_w"], dtype=np.float32).astype(bf16))
    arrs["fc2_w"] = np.ascontiguousarray(
        np.asarray(inputs["fc2_w"], dtype=np.float32).astype(bf16))
    for name, trivial in [
        ("ln0_g", 1.0), ("ln0_b", 0.0), ("ln1_g", 1.0), ("ln1_b", 0.0),
        ("lnf_g", 1.0), ("lnf_b", 0.0), ("fc1_b", 0.0), ("fc2_b", 0.0),
        ("a0_bq", 0.0), ("a0_bk", 0.0), ("a0_bv", 0.0), ("a0_bo", 0.0),
        ("a1_bq", 0.0), ("a1_bk", 0.0), ("a1_bv", 0.0), ("a1_bo", 0.0),
    ]:
        a = np.asarray(inputs[name], dtype=np.float32)
        if not np.all(a == trivial):
            cfg.add(name)
            arrs[name] = np.ascontiguousarray(a)
    return cfg, arrs


def kernel(**inputs):
    from concourse.bass_utils import run_bass_kernel_spmd

    cfg, arrs = _prep_inputs(inputs)
    key = frozenset(cfg)
    if key not in _CACHE:
        _CACHE[key] = _build_nc(key)
    nc = _CACHE[key]

    shared = {k: v for k, v in arrs.items() if k not in ("x0", "x1")}
    in_maps = []
    for b in range(N_CORES):
        m = dict(shared)
        m["x0"] = np.ascontiguousarray(arrs["x0"][b])
        m["x1"] = np.ascontiguousarray(arrs["x1"][b])
        in_maps.append(m)

    res = run_bass_kernel_spmd(nc, in_maps, core_ids=list(range(N_CORES)))
    out = np.stack([res.results[b]["out"] for b in range(N_CORES)], axis=0)
    return out.astype(np.float32)

